# revision 2
# baseline (speedup 1.0000x reference)
"""Trainium2 Bass kernel for nn_CustomActivation (v3).

    out[b, d] = sum_k alpha[k, d % 64] * relu(x[b, d] + gamma[k, d % 64])

Same engine assignment as v2 (bf16 I/O, DVE relus, PE diag-matmul MACs in
PSUM, single ACT eviction pass), plus:

  * block-granular DMA: one 2MB in-DMA and one 2MB out-DMA per 128-row
    block (10 DMA instrs/rep instead of 34)
  * block-granular DVE relus: FD=8192 (12 DVE instrs/rep instead of 48)
  * LDWEIGHTS dedup: consecutive InstLdweights with an identical weights
    AP are removed post-Tile (48 LDW/rep instead of 192) -- the PE weight
    register persists across matmuls, so only weight *changes* need a load
"""

import numpy as np
import ml_dtypes

import concourse.bacc as bacc
import concourse.mybir as mybir
from concourse.tile import TileContext

N_CORES = 8
B, D, L = 8192, 4096, 64
DS = D // N_CORES  # 512
P = 128
FT = 2048  # psum group columns (4 PSUM banks)
MMN = 512  # matmul moving free dim (1 PSUM bank)

BF16 = mybir.dt.bfloat16
F32 = mybir.dt.float32


def _ap_sig(pap) -> tuple:
    return (pap.memref, pap.offset, str(pap.ap), str(pap.dtype))


def dedup_ldweights(nc) -> int:
    """Remove InstLdweights whose weights AP matches the currently loaded
    weights (no intervening PE weight change). Returns #removed."""
    removed = 0
    passthrough = (
        mybir.InstMatmult,
        mybir.InstEventSemaphore,
        mybir.InstDrain,
    )
    for f in nc.m.functions:
        for blk in f.blocks:
            il = blk.instructions
            last_sig = None  # reset per block (conservative)
            drop = []
            for idx, ins in enumerate(il):
                if isinstance(ins, mybir.InstLdweights):
                    sig = _ap_sig(ins.ins[0])
                    if sig == last_sig:
                        drop.append(idx)
                        removed += 1
                    else:
                        last_sig = sig
                elif ins.engine == mybir.EngineType.PE and not isinstance(
                    ins, passthrough
                ):
                    last_sig = None
            for idx in reversed(drop):
                del il[idx]
    return removed


def build_program(ds: int = DS, b: int = B, n_rep: int = 1, dedup: bool = True):
    nc = bacc.Bacc("TRN2", target_bir_lowering=False, debug=False)

    xT = nc.dram_tensor("xT", [ds, b], BF16, kind="ExternalInput").ap()
    pv = nc.dram_tensor("pv", [P, 4], F32, kind="ExternalInput").ap()
    wd = nc.dram_tensor("wd", [P, 3 * P], BF16, kind="ExternalInput").ap()
    oT = nc.dram_tensor("oT", [ds, b], BF16, kind="ExternalOutput").ap()

    n_blk = ds // P  # 4
    n_f = b // FT  # 4
    n_j = FT // MMN  # 4
    A = mybir.AluOpType

    with TileContext(nc) as tc:
        with (
            tc.tile_pool(name="params", bufs=1) as ppool,
            tc.tile_pool(name="weights", bufs=1) as wpool,
            tc.tile_pool(name="xin", bufs=2) as xpool,
            tc.tile_pool(name="trelu", bufs=2) as tpool,
            tc.tile_pool(name="out", bufs=2) as opool,
            tc.tile_pool(name="psum", bufs=2, space="PSUM") as pspool,
        ):
            p_s = ppool.tile([P, 4], F32)
            nc.sync.dma_start(out=p_s, in_=pv)
            w_s = wpool.tile([P, 3 * P], BF16)
            nc.sync.dma_start(out=w_s, in_=wd)
            g = [p_s[:, k : k + 1] for k in range(3)]
            w = [w_s[:, k * P : (k + 1) * P] for k in range(3)]

            for _rep in range(n_rep):
                for blk in range(n_blk):
                    sl0 = slice(blk * P, (blk + 1) * P)
                    xt = xpool.tile([P, b], BF16)
                    nc.sync.dma_start(out=xt, in_=xT[sl0, :])
                    ts = [
                        tpool.tile([P, b], BF16, name=f"t{k}") for k in range(3)
                    ]
                    for k in range(3):
                        # DVE 4x: t_k = max(x + g_k, 0), whole block
                        nc.vector.tensor_scalar(ts[k], xt, g[k], 0.0, A.add, A.max)
                    ot = opool.tile([P, b], BF16)
                    for fi in range(n_f):
                        f0 = fi * FT
                        ps = pspool.tile([P, FT], F32)
                        for k in range(3):
                            for j in range(n_j):
                                sj = slice(f0 + j * MMN, f0 + (j + 1) * MMN)
                                nc.tensor.matmul(
                                    ps[:, j * MMN : (j + 1) * MMN],
                                    w[k],
                                    ts[k][:, sj],
                                    start=(k == 0),
                                    stop=(k == 2),
                                )
                        # ACT: evict psum -> bf16 slice of the block out tile
                        nc.scalar.copy(ot[:, f0 : f0 + FT], ps)
                    nc.sync.dma_start(out=oT[sl0, :], in_=ot)
    if dedup:
        dedup_ldweights(nc)
    nc.compile()
    return nc


def _param_vecs(gamma: np.ndarray) -> np.ndarray:
    g = np.tile(np.asarray(gamma, np.float32), (1, P // L))  # [3, 128]
    pv = np.stack([g[0], g[1], g[2], np.zeros(P, np.float32)], axis=1)
    return np.ascontiguousarray(pv.astype(np.float32))


def _weight_mats(alpha: np.ndarray) -> np.ndarray:
    a = np.tile(np.asarray(alpha, np.float32), (1, P // L))  # [3, 128]
    wd = np.zeros((P, 3 * P), np.float32)
    for k in range(3):
        wd[:, k * P : (k + 1) * P][np.arange(P), np.arange(P)] = a[k]
    return np.ascontiguousarray(wd.astype(ml_dtypes.bfloat16))


_program_cache: dict = {}


def kernel(x: np.ndarray, alpha: np.ndarray, gamma: np.ndarray) -> np.ndarray:
    from concourse.bass_utils import run_bass_kernel_spmd

    x = np.asarray(x, dtype=np.float32)
    pv = _param_vecs(gamma)
    wd = _weight_mats(alpha)

    xT = np.ascontiguousarray(x.T).astype(ml_dtypes.bfloat16)
    if "nc" not in _program_cache:
        _program_cache["nc"] = build_program()
    nc = _program_cache["nc"]
    in_maps = [
        {"xT": xT[c * DS : (c + 1) * DS], "pv": pv, "wd": wd}
        for c in range(N_CORES)
    ]
    res = run_bass_kernel_spmd(nc, in_maps, core_ids=list(range(N_CORES)))
    oT = np.concatenate([r["oT"] for r in res.results], axis=0)
    return np.ascontiguousarray(oT.T.astype(np.float32))
